# revision 3
# baseline (speedup 1.0000x reference)
"""Masked dense layer  out = tanh(x @ (w*mask_w) + b*mask_b)  on 8 TRN2 cores.

Data-parallel: x is sharded along the batch axis (32768 rows per core);
w/b/mask_w/mask_b are replicated. The v1 baseline (f32 HWDGE slabs + one DVE
AFFINE_MUL_REDUCE per row) was co-paced by DVE: AMR is a 1x-mode custom op
(~604ns per 512-elem row, 4/slab = 2.45us vs the 2.49us/slab DMA pace), which
stuttered the stream and left a ~9us DVE tail after the last slab landed.
A SWDGE cast-to-bf16 variant freed DVE but the cast-DMA path only streams at
~345 GB/s vs HWDGE's ~421 GB/s (97% of the 435 GB/s SBUF-fabric ceiling).

v3: HWDGE f32 slabs exclusively on the sync ring; per 4-row slab DVE does
rows 0-1 with AMR (604ns each) plus one f32 tensor_mul for rows 2-3 (1127ns),
and ACT reduces the two f32 product rows via activation(Copy, accum_out=...)
(~850ns each incl. accumulator read) then applies Tanh(+bias) per chunk.
DVE ~2.34us/slab, ACT ~1.9us/slab, both under the DMA pace, so the stream
free-runs. Params and the quarter-staged output writes ride the second HWDGE
ring (ACT sequencer) so the slab queue starts at t~0 and never stalls.
"""

import numpy as np

import concourse.bacc as bacc
import concourse.bass as bass
import concourse.tile as tile
from concourse import mybir
from concourse.bass_utils import run_bass_kernel_spmd

N, F = 262144, 512
C = 8                 # cores
R = N // C            # rows per core  = 32768
P = 128               # SBUF partitions
RP = R // P           # rows per partition = 256
T = 4                 # rows-per-partition per DMA slab (1 MiB per dma_start)
NCHUNK = RP // T      # 64 slabs per core
QUARTER = NCHUNK // 4

_cached_nc = None


def build_bass() -> bass.Bass:
    nc = bacc.Bacc()

    x = nc.declare_dram_parameter("x", [R, F], mybir.dt.float32, isOutput=False)
    w = nc.declare_dram_parameter("w", [F, 1], mybir.dt.float32, isOutput=False)
    b = nc.declare_dram_parameter("b", [1], mybir.dt.float32, isOutput=False)
    mask_w = nc.declare_dram_parameter(
        "mask_w", [F, 1], mybir.dt.int32, isOutput=False
    )
    mask_b = nc.declare_dram_parameter("mask_b", [1], mybir.dt.int32, isOutput=False)
    out = nc.declare_dram_parameter("out", [R, 1], mybir.dt.float32, isOutput=True)

    # partition p <- rows [p*RP, (p+1)*RP); per partition each slab is a
    # contiguous T*F*4 = 8 KiB DRAM run.
    x_r = x[:, :].rearrange("(p r) f -> p r f", p=P)      # [128, 256, 512]
    out_r = out[:, :].rearrange("(p r) one -> p (r one)", p=P)  # [128, 256]

    def bcast(src_handle, count):
        """DRAM AP replicating a contiguous `count`-element vector across P partitions."""
        ap = src_handle[:]
        return bass.AP(tensor=ap.tensor, offset=ap.offset, ap=[[0, P], [1, count]])

    def rep_mid(ap2d, k):
        """View a [P, F] SBUF AP as [P, k, F] with 0-stride middle dim."""
        return bass.AP(
            tensor=ap2d.tensor,
            offset=ap2d.offset,
            ap=[ap2d.ap[0], [0, k], ap2d.ap[1]],
        )

    with tile.TileContext(nc) as tc:
        with (
            tc.tile_pool(name="singles", bufs=1) as singles,
            tc.tile_pool(name="slabs", bufs=14) as slabs,
            tc.tile_pool(name="prods", bufs=3) as prods,
            tc.tile_pool(name="vjunk", bufs=2) as vjunk,
            tc.tile_pool(name="ajunk", bufs=2) as ajunk,
            tc.tile_pool(name="stages", bufs=4) as stages,
        ):
            # masked weights, broadcast to all partitions: wm[p, f] = w[f]*mask_w[f]
            # (second HWDGE ring - ACT sequencer - so the slab queue is untouched)
            wb = singles.tile([P, F], mybir.dt.float32)
            nc.scalar.dma_start(out=wb, in_=bcast(w, F))
            mwi = singles.tile([P, F], mybir.dt.int32)
            nc.scalar.dma_start(out=mwi, in_=bcast(mask_w, F))
            mw = singles.tile([P, F], mybir.dt.float32)
            nc.vector.tensor_copy(mw, mwi)  # i32 -> f32
            wm = singles.tile([P, F], mybir.dt.float32)
            nc.vector.tensor_mul(wm, wb, mw)

            # masked bias, per-partition scalar: bm[p, 0] = b[0]*mask_b[0]
            bb = singles.tile([P, 1], mybir.dt.float32)
            nc.scalar.dma_start(out=bb, in_=bcast(b, 1))
            mbi = singles.tile([P, 1], mybir.dt.int32)
            nc.scalar.dma_start(out=mbi, in_=bcast(mask_b, 1))
            mb = singles.tile([P, 1], mybir.dt.float32)
            nc.vector.tensor_copy(mb, mbi)  # i32 -> f32
            bm = singles.tile([P, 1], mybir.dt.float32)
            nc.vector.tensor_mul(bm, bb, mb)

            outt = singles.tile([P, RP], mybir.dt.float32)
            for c in range(NCHUNK):
                slab = slabs.tile([P, T, F], mybir.dt.float32, tag="slab")
                nc.sync.dma_start(out=slab, in_=x_r[:, c * T : (c + 1) * T, :])
                stage = stages.tile([P, T], mybir.dt.float32, tag="stage")
                # rows 0-1: fused mul+reduce on DVE
                for t in range(2):
                    junk = vjunk.tile([P, F], mybir.dt.bfloat16, tag="vj")
                    nc.vector.affine_mul_reduce(
                        out=junk,
                        accum_out=stage[:, t : t + 1],
                        in0=slab[:, t, :],
                        in1=wm,
                        scale=1.0,
                        bias=0.0,
                    )
                # rows 2-3: f32 multiply on DVE, reduce on ACT
                prod = prods.tile([P, 2, F], mybir.dt.float32, tag="prod")
                nc.vector.tensor_mul(prod, slab[:, 2:4, :], rep_mid(wm[:, :], 2))
                for t in range(2):
                    aj = ajunk.tile([P, F], mybir.dt.bfloat16, tag="aj")
                    nc.scalar.activation(
                        out=aj,
                        in_=prod[:, t, :],
                        func=mybir.ActivationFunctionType.Copy,
                        accum_out=stage[:, 2 + t : 3 + t],
                    )
                nc.scalar.activation(
                    out=outt[:, c * T : (c + 1) * T],
                    in_=stage,
                    func=mybir.ActivationFunctionType.Tanh,
                    bias=bm,
                    scale=1.0,
                )
                # stream the finished quarters out on the second HWDGE ring
                if (c + 1) % QUARTER == 0 and c + 1 < NCHUNK:
                    q0 = (c + 1 - QUARTER) * T
                    q1 = (c + 1) * T
                    nc.scalar.dma_start(
                        out=out_r[:, q0:q1], in_=outt[:, q0:q1]
                    )
            q0 = (NCHUNK - QUARTER) * T
            nc.scalar.dma_start(out=out_r[:, q0:], in_=outt[:, q0:])

    nc.finalize()
    return nc


def run_sharded(inputs: dict, **run_kwargs):
    """Shard inputs, run on 8 cores, gather. Returns (output, BassKernelResults)."""
    global _cached_nc
    if _cached_nc is None:
        _cached_nc = build_bass()
    nc = _cached_nc

    x = np.ascontiguousarray(np.asarray(inputs["x"], dtype=np.float32))
    w = np.ascontiguousarray(np.asarray(inputs["w"], dtype=np.float32))
    b = np.ascontiguousarray(np.asarray(inputs["b"], dtype=np.float32))
    mask_w = np.ascontiguousarray(np.asarray(inputs["mask_w"], dtype=np.int32))
    mask_b = np.ascontiguousarray(np.asarray(inputs["mask_b"], dtype=np.int32))

    in_maps = [
        {
            "x": x[i * R : (i + 1) * R],
            "w": w,
            "b": b,
            "mask_w": mask_w,
            "mask_b": mask_b,
        }
        for i in range(C)
    ]
    res = run_bass_kernel_spmd(nc, in_maps, core_ids=list(range(C)), **run_kwargs)
    outs = [res.results[i]["out"] for i in range(C)]
    return np.concatenate(outs, axis=0), res


def kernel(x, w, b, mask_w, mask_b) -> np.ndarray:
    out, _ = run_sharded(
        {"x": x, "w": w, "b": b, "mask_w": mask_w, "mask_b": mask_b}
    )
    return out


# revision 4
# speedup vs baseline: 1.0497x; 1.0497x over previous
"""Masked dense layer  out = tanh(x @ (w*mask_w) + b*mask_b)  on 8 TRN2 cores.

Data-parallel: x is sharded along the batch axis (32768 rows per core);
w/b/mask_w/mask_b are replicated. The v1 baseline (f32 HWDGE slabs + one DVE
AFFINE_MUL_REDUCE per row) was co-paced by DVE: AMR is a 1x-mode custom op
(~604ns per 512-elem row, ~2.45us/slab vs the ~2.5us/slab DMA pace), which
stuttered the stream and left a ~9us DVE tail after the last slab landed.

v4 rebalances compute across DVE and ACT so the HWDGE f32 stream (measured
~421 GB/s within-busy, 97% of the 435 GB/s SBUF-fabric ceiling) free-runs:
- slabs stream alone on the sync HWDGE ring (1 MiB per dma_start, 14-deep);
- per 4-row slab DVE does rows 0-1 with AMR (604ns each) plus one f32
  tensor_mul for rows 2-3 (1137ns); ACT reduces the two product rows via
  activation(Copy, accum_out=...) (~850ns each) and applies Tanh(+bias);
- params ride the otherwise-idle gpsimd SWDGE ring (which also casts the
  int32 masks to f32 in-flight), so the slab queue starts at t~0;
- output is a single DMA at the end (mid-stream output DMAs were measured
  to stall the slab stream ~5us each via shared DMA-completion sem lanes).
"""

import numpy as np

import concourse.bacc as bacc
import concourse.bass as bass
import concourse.tile as tile
from concourse import mybir
from concourse.bass_utils import run_bass_kernel_spmd

N, F = 262144, 512
C = 8                 # cores
R = N // C            # rows per core  = 32768
P = 128               # SBUF partitions
RP = R // P           # rows per partition = 256
T = 4                 # rows-per-partition per DMA slab (1 MiB per dma_start)
NCHUNK = RP // T      # 64 slabs per core

_cached_nc = None


def build_bass() -> bass.Bass:
    nc = bacc.Bacc()

    x = nc.declare_dram_parameter("x", [R, F], mybir.dt.float32, isOutput=False)
    w = nc.declare_dram_parameter("w", [F, 1], mybir.dt.float32, isOutput=False)
    b = nc.declare_dram_parameter("b", [1], mybir.dt.float32, isOutput=False)
    mask_w = nc.declare_dram_parameter(
        "mask_w", [F, 1], mybir.dt.int32, isOutput=False
    )
    mask_b = nc.declare_dram_parameter("mask_b", [1], mybir.dt.int32, isOutput=False)
    out = nc.declare_dram_parameter("out", [R, 1], mybir.dt.float32, isOutput=True)

    # partition p <- rows [p*RP, (p+1)*RP); per partition each slab is a
    # contiguous T*F*4 = 8 KiB DRAM run.
    x_r = x[:, :].rearrange("(p r) f -> p r f", p=P)      # [128, 256, 512]
    out_r = out[:, :].rearrange("(p r) one -> p (r one)", p=P)  # [128, 256]

    def bcast(src_handle, count):
        """DRAM AP replicating a contiguous `count`-element vector across P partitions."""
        ap = src_handle[:]
        return bass.AP(tensor=ap.tensor, offset=ap.offset, ap=[[0, P], [1, count]])

    def rep_mid(ap2d, k):
        """View a [P, F] SBUF AP as [P, k, F] with 0-stride middle dim."""
        return bass.AP(
            tensor=ap2d.tensor,
            offset=ap2d.offset,
            ap=[ap2d.ap[0], [0, k], ap2d.ap[1]],
        )

    with tile.TileContext(nc) as tc:
        with (
            tc.tile_pool(name="singles", bufs=1) as singles,
            tc.tile_pool(name="slabs", bufs=14) as slabs,
            tc.tile_pool(name="prods", bufs=4) as prods,
            tc.tile_pool(name="vjunk", bufs=3) as vjunk,
            tc.tile_pool(name="ajunk", bufs=3) as ajunk,
            tc.tile_pool(name="stages", bufs=6) as stages,
        ):
            # masked weights, broadcast to all partitions: wm[p, f] = w[f]*mask_w[f]
            # (gpsimd SWDGE ring: leaves the sync ring free and casts i32->f32
            # in-flight, so no DVE cast ops are needed)
            wb = singles.tile([P, F], mybir.dt.float32)
            nc.gpsimd.dma_start(out=wb, in_=bcast(w, F))
            mw = singles.tile([P, F], mybir.dt.float32)
            nc.gpsimd.dma_start(out=mw, in_=bcast(mask_w, F))
            wm = singles.tile([P, F], mybir.dt.float32)
            nc.vector.tensor_mul(wm, wb, mw)

            # masked bias, per-partition scalar: bm[p, 0] = b[0]*mask_b[0]
            bb = singles.tile([P, 1], mybir.dt.float32)
            nc.gpsimd.dma_start(out=bb, in_=bcast(b, 1))
            mb = singles.tile([P, 1], mybir.dt.float32)
            nc.gpsimd.dma_start(out=mb, in_=bcast(mask_b, 1))
            bm = singles.tile([P, 1], mybir.dt.float32)
            nc.vector.tensor_mul(bm, bb, mb)

            outt = singles.tile([P, RP], mybir.dt.float32)
            for c in range(NCHUNK):
                slab = slabs.tile([P, T, F], mybir.dt.float32, tag="slab")
                nc.sync.dma_start(out=slab, in_=x_r[:, c * T : (c + 1) * T, :])
                stage = stages.tile([P, T], mybir.dt.float32, tag="stage")
                # rows 0-1: fused mul+reduce on DVE
                for t in range(2):
                    junk = vjunk.tile([P, F], mybir.dt.bfloat16, tag="vj")
                    nc.vector.affine_mul_reduce(
                        out=junk,
                        accum_out=stage[:, t : t + 1],
                        in0=slab[:, t, :],
                        in1=wm,
                        scale=1.0,
                        bias=0.0,
                    )
                # rows 2-3: f32 multiply on DVE, reduce on ACT
                prod = prods.tile([P, 2, F], mybir.dt.float32, tag="prod")
                nc.vector.tensor_mul(prod, slab[:, 2:4, :], rep_mid(wm[:, :], 2))
                for t in range(2):
                    aj = ajunk.tile([P, F], mybir.dt.bfloat16, tag="aj")
                    nc.scalar.activation(
                        out=aj,
                        in_=prod[:, t, :],
                        func=mybir.ActivationFunctionType.Copy,
                        accum_out=stage[:, 2 + t : 3 + t],
                    )
                nc.scalar.activation(
                    out=outt[:, c * T : (c + 1) * T],
                    in_=stage,
                    func=mybir.ActivationFunctionType.Tanh,
                    bias=bm,
                    scale=1.0,
                )
            nc.sync.dma_start(out=out_r, in_=outt)

    nc.finalize()
    return nc


def run_sharded(inputs: dict, **run_kwargs):
    """Shard inputs, run on 8 cores, gather. Returns (output, BassKernelResults)."""
    global _cached_nc
    if _cached_nc is None:
        _cached_nc = build_bass()
    nc = _cached_nc

    x = np.ascontiguousarray(np.asarray(inputs["x"], dtype=np.float32))
    w = np.ascontiguousarray(np.asarray(inputs["w"], dtype=np.float32))
    b = np.ascontiguousarray(np.asarray(inputs["b"], dtype=np.float32))
    mask_w = np.ascontiguousarray(np.asarray(inputs["mask_w"], dtype=np.int32))
    mask_b = np.ascontiguousarray(np.asarray(inputs["mask_b"], dtype=np.int32))

    in_maps = [
        {
            "x": x[i * R : (i + 1) * R],
            "w": w,
            "b": b,
            "mask_w": mask_w,
            "mask_b": mask_b,
        }
        for i in range(C)
    ]
    res = run_bass_kernel_spmd(nc, in_maps, core_ids=list(range(C)), **run_kwargs)
    outs = [res.results[i]["out"] for i in range(C)]
    return np.concatenate(outs, axis=0), res


def kernel(x, w, b, mask_w, mask_b) -> np.ndarray:
    out, _ = run_sharded(
        {"x": x, "w": w, "b": b, "mask_w": mask_w, "mask_b": mask_b}
    )
    return out


# revision 11
# speedup vs baseline: 1.1719x; 1.1165x over previous
"""Masked dense layer  out = tanh(x @ (w*mask_w) + b*mask_b)  on 8 TRN2 cores.

Data-parallel: x is sharded along the batch axis (32768 rows per core);
w/b/mask_w/mask_b are replicated. The HWDGE f32 slab stream runs at
~411-421 GB/s per core (vs the 435 GB/s SBUF-fabric ceiling; the 64 MiB/core
HBM read is mandatory traffic), so the kernel is built so that stream
free-runs and everything else hides behind it:

- Work per slab is split across DVE and ACT so neither engine paces the
  stream: DVE does half the rows with AFFINE_MUL_REDUCE (a 1x custom op,
  ~604ns/row) plus one f32->bf16 tensor_mul for the other half; ACT reduces
  the product rows via activation(Copy, accum_out=...) (~850ns/row) and
  applies Tanh(+bias) per chunk. (The v1 baseline ran all rows through AMR
  on DVE, which co-paced the stream and added a ~9us tail.)
- Chunk schedule [1,1,2, 4x62, 2,2]: tiny leading chunks because a DMA's
  completion semaphore lands ~5us after the bytes under a saturated fabric -
  small first slabs get DVE computing by ~14us instead of ~18us; 1 MiB
  middle slabs are the measured-fastest HWDGE shape (~2.5us/MiB; 2 MiB
  slabs measured 18% slower per byte); small tail chunks shorten the final
  dependency chain.
- Params load on the sync ring ahead of the slabs (issued later, their sems
  don't fire for ~10us); param math runs on DVE before chunk 0's slab sem
  arrives (GpSimd tensor ops trigger an 8us ucode LIBRARY_RELOAD mid-stream).
- No mid-stream output DMAs (they stall the slab stream ~5us each via
  shared DMA-completion semaphore lanes): one body write issued after all
  slab dma_starts, and the last 8 rows after the final Tanh.
"""

import numpy as np

import concourse.bacc as bacc
import concourse.bass as bass
import concourse.tile as tile
from concourse import mybir
from concourse.bass_utils import run_bass_kernel_spmd

N, F = 262144, 512
C = 8                 # cores
R = N // C            # rows per core  = 32768
P = 128               # SBUF partitions
RP = R // P           # rows per partition = 256
CHUNKS = [1, 1, 2] + [4] * 62 + [2, 2]
assert sum(CHUNKS) == RP

_cached_nc = None


def build_bass() -> bass.Bass:
    nc = bacc.Bacc()

    x = nc.declare_dram_parameter("x", [R, F], mybir.dt.float32, isOutput=False)
    w = nc.declare_dram_parameter("w", [F, 1], mybir.dt.float32, isOutput=False)
    b = nc.declare_dram_parameter("b", [1], mybir.dt.float32, isOutput=False)
    mask_w = nc.declare_dram_parameter(
        "mask_w", [F, 1], mybir.dt.int32, isOutput=False
    )
    mask_b = nc.declare_dram_parameter("mask_b", [1], mybir.dt.int32, isOutput=False)
    out = nc.declare_dram_parameter("out", [R, 1], mybir.dt.float32, isOutput=True)

    # partition p <- rows [p*RP, (p+1)*RP)
    x_r = x[:, :].rearrange("(p r) f -> p r f", p=P)      # [128, 256, 512]
    out_r = out[:, :].rearrange("(p r) one -> p (r one)", p=P)  # [128, 256]

    def bcast(src_handle, count):
        """DRAM AP replicating a contiguous `count`-element vector across P partitions."""
        ap = src_handle[:]
        return bass.AP(tensor=ap.tensor, offset=ap.offset, ap=[[0, P], [1, count]])

    def rep_mid(ap2d, k):
        """View a [P, F] SBUF AP as [P, k, F] with 0-stride middle dim."""
        return bass.AP(
            tensor=ap2d.tensor,
            offset=ap2d.offset,
            ap=[ap2d.ap[0], [0, k], ap2d.ap[1]],
        )

    with tile.TileContext(nc) as tc:
        with (
            tc.tile_pool(name="singles", bufs=1) as singles,
            tc.tile_pool(name="slabs_big", bufs=13) as slabs_big,
            tc.tile_pool(name="slabs_small", bufs=2) as slabs_small,
            tc.tile_pool(name="prods", bufs=4) as prods,
            tc.tile_pool(name="vjunk", bufs=3) as vjunk,
            tc.tile_pool(name="ajunk", bufs=3) as ajunk,
            tc.tile_pool(name="stages", bufs=3) as stages,
        ):
            # param loads ride the sync ring AHEAD of the slab stream (~2.5us)
            wb = singles.tile([P, F], mybir.dt.float32)
            nc.sync.dma_start(out=wb, in_=bcast(w, F))
            mwi = singles.tile([P, F], mybir.dt.int32)
            nc.sync.dma_start(out=mwi, in_=bcast(mask_w, F))
            bb = singles.tile([P, 1], mybir.dt.float32)
            nc.sync.dma_start(out=bb, in_=bcast(b, 1))
            mbi = singles.tile([P, 1], mybir.dt.int32)
            nc.sync.dma_start(out=mbi, in_=bcast(mask_b, 1))

            # wm on DVE right away (ready before chunk 0's slab sem arrives)
            mw = singles.tile([P, F], mybir.dt.float32)
            nc.vector.tensor_copy(mw, mwi)  # i32 -> f32
            wm = singles.tile([P, F], mybir.dt.float32)
            nc.vector.tensor_mul(wm, wb, mw)

            mb = singles.tile([P, 1], mybir.dt.float32)
            bm = singles.tile([P, 1], mybir.dt.float32)

            outt = singles.tile([P, RP], mybir.dt.float32)
            r0 = 0
            for ci, tr in enumerate(CHUNKS):
                half = tr // 2
                pool = slabs_big if tr == 4 else slabs_small
                slab = pool.tile([P, tr, F], mybir.dt.float32, tag=f"slab{tr}")
                nc.sync.dma_start(out=slab, in_=x_r[:, r0 : r0 + tr, :])
                stage = stages.tile([P, tr], mybir.dt.float32, tag=f"stage{tr}")
                # first half (rounded up) of the rows: fused mul+reduce on DVE
                n_amr = tr - half
                for t in range(n_amr):
                    junk = vjunk.tile([P, F], mybir.dt.bfloat16, tag="vj")
                    nc.vector.affine_mul_reduce(
                        out=junk,
                        accum_out=stage[:, t : t + 1],
                        in0=slab[:, t, :],
                        in1=wm,
                        scale=1.0,
                        bias=0.0,
                    )
                # second half: f32 multiply (bf16 product) on DVE, reduce on ACT
                if half:
                    prod = prods.tile(
                        [P, half, F], mybir.dt.bfloat16, tag=f"prod{half}"
                    )
                    nc.vector.tensor_mul(
                        prod, slab[:, n_amr:tr, :], rep_mid(wm[:, :], half)
                    )
                    for t in range(half):
                        aj = ajunk.tile([P, F], mybir.dt.bfloat16, tag="aj")
                        nc.scalar.activation(
                            out=aj,
                            in_=prod[:, t, :],
                            func=mybir.ActivationFunctionType.Copy,
                            accum_out=stage[:, n_amr + t : n_amr + t + 1],
                        )
                nc.scalar.activation(
                    out=outt[:, r0 : r0 + tr],
                    in_=stage,
                    func=mybir.ActivationFunctionType.Tanh,
                    bias=bm,
                    scale=1.0,
                )
                r0 += tr
                if ci == 0:
                    # bias prep on DVE, squeezed in after chunk 0's ops
                    # (first Tanh may wait ~0.3us on bm; nothing downstream
                    # needs it earlier)
                    nc.vector.tensor_copy(mb, mbi)  # i32 -> f32
                    nc.vector.tensor_mul(bm, bb, mb)
            # issued after every slab dma_start (the sync ring is FIFO): the
            # body write drains while the tail chunks compute; the final 8
            # rows follow the last Tanh.
            nc.sync.dma_start(out=out_r[:, : RP - 4], in_=outt[:, : RP - 4])
            nc.sync.dma_start(out=out_r[:, RP - 4 :], in_=outt[:, RP - 4 :])

    nc.finalize()
    return nc


def run_sharded(inputs: dict, **run_kwargs):
    """Shard inputs, run on 8 cores, gather. Returns (output, BassKernelResults)."""
    global _cached_nc
    if _cached_nc is None:
        _cached_nc = build_bass()
    nc = _cached_nc

    x = np.ascontiguousarray(np.asarray(inputs["x"], dtype=np.float32))
    w = np.ascontiguousarray(np.asarray(inputs["w"], dtype=np.float32))
    b = np.ascontiguousarray(np.asarray(inputs["b"], dtype=np.float32))
    mask_w = np.ascontiguousarray(np.asarray(inputs["mask_w"], dtype=np.int32))
    mask_b = np.ascontiguousarray(np.asarray(inputs["mask_b"], dtype=np.int32))

    in_maps = [
        {
            "x": x[i * R : (i + 1) * R],
            "w": w,
            "b": b,
            "mask_w": mask_w,
            "mask_b": mask_b,
        }
        for i in range(C)
    ]
    res = run_bass_kernel_spmd(nc, in_maps, core_ids=list(range(C)), **run_kwargs)
    outs = [res.results[i]["out"] for i in range(C)]
    return np.concatenate(outs, axis=0), res


def kernel(x, w, b, mask_w, mask_b) -> np.ndarray:
    out, _ = run_sharded(
        {"x": x, "w": w, "b": b, "mask_w": mask_w, "mask_b": mask_b}
    )
    return out


# revision 19
# speedup vs baseline: 1.1834x; 1.0097x over previous
"""Masked dense layer  out = tanh(x @ (w*mask_w) + b*mask_b)  on 8 TRN2 cores.

Data-parallel: x is sharded along the batch axis (32768 rows per core);
w/b/mask_w/mask_b are replicated. The HWDGE f32 slab stream runs at
~411-421 GB/s per core (vs the 435 GB/s SBUF-fabric ceiling; the 64 MiB/core
HBM read is mandatory traffic), so the kernel is built so that stream
free-runs and everything else hides behind it:

- Work per slab is split across DVE and ACT so neither engine paces the
  stream: DVE does half the rows with AFFINE_MUL_REDUCE (a 1x custom op,
  ~604ns/row) plus one f32 tensor_mul for the other half; ACT reduces the
  product rows via activation(Copy, accum_out=...) (~850ns/row) and
  applies Tanh(+bias) per chunk. (The v1 baseline ran all rows through AMR
  on DVE, which co-paced the stream and added a ~9us tail. An alternating
  1-AMR/3-ACT split measured 9us slower: the 3-row ACT bursts exceed the
  per-slab pace.)
- Chunk schedule [1,1,2, 4x62, 2,1,1]: tiny leading chunks because a DMA's
  completion semaphore lands ~5us after the bytes under a saturated fabric -
  small first slabs get DVE computing by ~14us instead of ~18us; 1 MiB
  middle slabs are the measured-fastest HWDGE shape (~2.5us/MiB; 2 MiB
  slabs measured 18% slower per byte); small tail chunks shorten the final
  dependency chain.
- Params load on the sync ring ahead of the slabs (issued later, their sems
  don't fire for ~10us); param math runs on DVE before chunk 0's slab sem
  arrives (GpSimd tensor ops trigger an 8us ucode LIBRARY_RELOAD mid-stream).
- No mid-stream output DMAs (they stall the slab stream ~5us each via
  shared DMA-completion semaphore lanes): one body write issued after all
  slab dma_starts, and the last 8 rows after the final Tanh.
"""

import numpy as np

import concourse.bacc as bacc
import concourse.bass as bass
import concourse.tile as tile
from concourse import mybir
from concourse.bass_utils import run_bass_kernel_spmd

N, F = 262144, 512
C = 8                 # cores
R = N // C            # rows per core  = 32768
P = 128               # SBUF partitions
RP = R // P           # rows per partition = 256
CHUNKS = [1, 1, 2] + [4] * 62 + [2, 1, 1]
assert sum(CHUNKS) == RP

_cached_nc = None


def build_bass() -> bass.Bass:
    nc = bacc.Bacc()

    x = nc.declare_dram_parameter("x", [R, F], mybir.dt.float32, isOutput=False)
    w = nc.declare_dram_parameter("w", [F, 1], mybir.dt.float32, isOutput=False)
    b = nc.declare_dram_parameter("b", [1], mybir.dt.float32, isOutput=False)
    mask_w = nc.declare_dram_parameter(
        "mask_w", [F, 1], mybir.dt.int32, isOutput=False
    )
    mask_b = nc.declare_dram_parameter("mask_b", [1], mybir.dt.int32, isOutput=False)
    out = nc.declare_dram_parameter("out", [R, 1], mybir.dt.float32, isOutput=True)

    # partition p <- rows [p*RP, (p+1)*RP)
    x_r = x[:, :].rearrange("(p r) f -> p r f", p=P)      # [128, 256, 512]
    out_r = out[:, :].rearrange("(p r) one -> p (r one)", p=P)  # [128, 256]

    def bcast(src_handle, count):
        """DRAM AP replicating a contiguous `count`-element vector across P partitions."""
        ap = src_handle[:]
        return bass.AP(tensor=ap.tensor, offset=ap.offset, ap=[[0, P], [1, count]])

    def rep_mid(ap2d, k):
        """View a [P, F] SBUF AP as [P, k, F] with 0-stride middle dim."""
        return bass.AP(
            tensor=ap2d.tensor,
            offset=ap2d.offset,
            ap=[ap2d.ap[0], [0, k], ap2d.ap[1]],
        )

    with tile.TileContext(nc) as tc:
        with (
            tc.tile_pool(name="singles", bufs=1) as singles,
            tc.tile_pool(name="slabs_big", bufs=13) as slabs_big,
            tc.tile_pool(name="slabs_small", bufs=2) as slabs_small,
            tc.tile_pool(name="prods", bufs=4) as prods,
            tc.tile_pool(name="vjunk", bufs=3) as vjunk,
            tc.tile_pool(name="ajunk", bufs=3) as ajunk,
            tc.tile_pool(name="stages", bufs=3) as stages,
        ):
            # param loads ride the sync ring AHEAD of the slab stream (~2.5us)
            wb = singles.tile([P, F], mybir.dt.float32)
            nc.sync.dma_start(out=wb, in_=bcast(w, F))
            mwi = singles.tile([P, F], mybir.dt.int32)
            nc.sync.dma_start(out=mwi, in_=bcast(mask_w, F))
            bb = singles.tile([P, 1], mybir.dt.float32)
            nc.sync.dma_start(out=bb, in_=bcast(b, 1))
            mbi = singles.tile([P, 1], mybir.dt.int32)
            nc.sync.dma_start(out=mbi, in_=bcast(mask_b, 1))

            # wm on DVE right away (ready before chunk 0's slab sem arrives)
            mw = singles.tile([P, F], mybir.dt.float32)
            nc.vector.tensor_copy(mw, mwi)  # i32 -> f32
            wm = singles.tile([P, F], mybir.dt.float32)
            nc.vector.tensor_mul(wm, wb, mw)

            mb = singles.tile([P, 1], mybir.dt.float32)
            bm = singles.tile([P, 1], mybir.dt.float32)

            outt = singles.tile([P, RP], mybir.dt.float32)
            r0 = 0
            for ci, tr in enumerate(CHUNKS):
                half = tr // 2
                pool = slabs_big if tr == 4 else slabs_small
                slab = pool.tile([P, tr, F], mybir.dt.float32, tag=f"slab{tr}")
                nc.sync.dma_start(out=slab, in_=x_r[:, r0 : r0 + tr, :])
                stage = stages.tile([P, tr], mybir.dt.float32, tag=f"stage{tr}")
                # first half (rounded up) of the rows: fused mul+reduce on DVE
                n_amr = tr - half
                for t in range(n_amr):
                    junk = vjunk.tile([P, F], mybir.dt.bfloat16, tag="vj")
                    nc.vector.affine_mul_reduce(
                        out=junk,
                        accum_out=stage[:, t : t + 1],
                        in0=slab[:, t, :],
                        in1=wm,
                        scale=1.0,
                        bias=0.0,
                    )
                # second half: f32 multiply on DVE, reduce on ACT (f32 product:
                # same measured op costs as bf16, keeps rel err at ~6e-7)
                if half:
                    prod = prods.tile(
                        [P, half, F], mybir.dt.float32, tag=f"prod{half}"
                    )
                    nc.vector.tensor_mul(
                        prod, slab[:, n_amr:tr, :], rep_mid(wm[:, :], half)
                    )
                    for t in range(half):
                        aj = ajunk.tile([P, F], mybir.dt.bfloat16, tag="aj")
                        nc.scalar.activation(
                            out=aj,
                            in_=prod[:, t, :],
                            func=mybir.ActivationFunctionType.Copy,
                            accum_out=stage[:, n_amr + t : n_amr + t + 1],
                        )
                nc.scalar.activation(
                    out=outt[:, r0 : r0 + tr],
                    in_=stage,
                    func=mybir.ActivationFunctionType.Tanh,
                    bias=bm,
                    scale=1.0,
                )
                r0 += tr
                if ci == 0:
                    # bias prep on DVE, squeezed in after chunk 0's ops
                    # (first Tanh may wait ~0.3us on bm; nothing downstream
                    # needs it earlier)
                    nc.vector.tensor_copy(mb, mbi)  # i32 -> f32
                    nc.vector.tensor_mul(bm, bb, mb)
            # issued after every slab dma_start (the sync ring is FIFO): the
            # body write drains while the tail chunks compute; the final 8
            # rows follow the last Tanh.
            nc.sync.dma_start(out=out_r[:, : RP - 4], in_=outt[:, : RP - 4])
            nc.sync.dma_start(out=out_r[:, RP - 4 :], in_=outt[:, RP - 4 :])

    nc.finalize()
    return nc


def run_sharded(inputs: dict, **run_kwargs):
    """Shard inputs, run on 8 cores, gather. Returns (output, BassKernelResults)."""
    global _cached_nc
    if _cached_nc is None:
        _cached_nc = build_bass()
    nc = _cached_nc

    x = np.ascontiguousarray(np.asarray(inputs["x"], dtype=np.float32))
    w = np.ascontiguousarray(np.asarray(inputs["w"], dtype=np.float32))
    b = np.ascontiguousarray(np.asarray(inputs["b"], dtype=np.float32))
    mask_w = np.ascontiguousarray(np.asarray(inputs["mask_w"], dtype=np.int32))
    mask_b = np.ascontiguousarray(np.asarray(inputs["mask_b"], dtype=np.int32))

    in_maps = [
        {
            "x": x[i * R : (i + 1) * R],
            "w": w,
            "b": b,
            "mask_w": mask_w,
            "mask_b": mask_b,
        }
        for i in range(C)
    ]
    res = run_bass_kernel_spmd(nc, in_maps, core_ids=list(range(C)), **run_kwargs)
    outs = [res.results[i]["out"] for i in range(C)]
    return np.concatenate(outs, axis=0), res


def kernel(x, w, b, mask_w, mask_b) -> np.ndarray:
    out, _ = run_sharded(
        {"x": x, "w": w, "b": b, "mask_w": mask_w, "mask_b": mask_b}
    )
    return out
